# revision 11
# baseline (speedup 1.0000x reference)
"""BinaryConv2D forward on 8 Trainium2 NeuronCores.

out = conv2d_same(inputs, sign(clip(kernel)))   (NHWC, HWIO, 3x3, stride 1)

Sharding: data-parallel over batch (32 images -> 4 per core); the binarized
3x3x256x256 kernel is replicated (forward only, no gradient collective).

Strategy (v2 — fp8 DoubleRow):
  - Weights are +-1 after sign(), exactly representable in fp8e4. The input
    is split on the host into hi = fp8e4(x), lo = fp8e4(x - hi); two fp8
    passes reproduce ~7e-4 relative accuracy (gate is 2e-2) because the
    residual quantization error is (2^-4)^2 of |x|.
  - fp8e4 enables MatmulPerfMode.DoubleRow: the PE virtualizes to 128x256,
    contracting all 256 input channels in ONE matmul (lhsT [128,2,128],
    rhs [128,2,N]) at 2 MACs/cell/cycle — 4x fewer PE cycles than the bf16
    hi/lo baseline for the conv stream.
  - The host also pre-transposes each image to channel-major and embeds the
    SAME-padding in a flat padded layout: 58 rows x 58 cols (+1 guard elem at
    each end, 3366 per partition), zeros at the borders. Every conv tap
    (dy,dx) then reads one CONTIGUOUS shifted window of length 464 covering
    8 output rows — no on-device transposes, no per-tap row clipping, and
    the upload is 2 fp8 bytes/element instead of 4 (fp32).
  - Per (img, oc-half, 8-row block): one PSUM group of 18 accumulating
    DoubleRow matmuls (9 taps x {hi,lo}), out free = 464 fp32 (fits a 2KB
    bank). DVE evicts the 56 valid columns/row to bf16; the result is stored
    channel-major [oc, 128, 3136] and un-transposed to NHWC fp32 on the host.
  - ~36 warmup matmuls at t=0 cover the PE clock ramp while the weights and
    the first image chunks load.
"""

import numpy as np

P = 128
H = 56
W = 56
C = 256
XP = W + 1                   # padded row width (57): one zero col at x=-1.
                             # Reading one past the right edge lands on the
                             # NEXT row's pad col (also zero), so a single
                             # pad col covers both SAME-padding sides.
YP = H + 2                   # padded rows (58)
FLAT = YP * XP + 2           # flat padded image + 1 guard elem at each end
NCORES = 8
NTOT = 32
NI = NTOT // NCORES          # images per core
NPIX = H * W                 # 3136
RB = 8                       # output rows per psum block
NT = H // RB                 # 7 psum blocks
NWARM = 36                   # PE clock-ramp warmup matmuls

_cache = {}


def _flat(y, x):
    # flat index of padded coord (row y in 0..57, col x in 0..57)
    return 1 + y * XP + x


def _build_bass(ni=NI, loops=1):
    import concourse.bacc as bacc
    import concourse.mybir as mybir
    import concourse.tile as tile
    from concourse.masks import make_identity
    from contextlib import ExitStack

    f32 = mybir.dt.float32
    bf16 = mybir.dt.bfloat16
    f8 = mybir.dt.float8e4
    DR = mybir.MatmulPerfMode.DoubleRow

    nc = bacc.Bacc()
    # [img, tag(hi/lo), cc, cin_p, flat] fp8, host-padded (borders zero)
    xq = nc.dram_tensor("xq", [ni, 2, 2, P, FLAT], f8, kind="ExternalInput")
    # [cin_p, tap, cc, cout] fp8 sign weights, host-binarized
    wq = nc.dram_tensor("wq", [P, 9, 2, C], f8, kind="ExternalInput")
    # channel-major bf16 output; host un-transposes to NHWC fp32
    y = nc.dram_tensor("y", [ni, 2, P, NPIX], bf16, kind="ExternalOutput")

    TAPS = [(ky, kx) for ky in range(3) for kx in range(3)]

    with ExitStack() as ctx:
        tc = ctx.enter_context(tile.TileContext(nc))
        const = ctx.enter_context(tc.tile_pool(name="const", bufs=1))
        wpool = ctx.enter_context(tc.tile_pool(name="wpool", bufs=1))
        xpool = ctx.enter_context(tc.tile_pool(name="xpool", bufs=2))
        outp = ctx.enter_context(tc.tile_pool(name="outp", bufs=3))
        psc = ctx.enter_context(tc.tile_pool(name="psc", bufs=3, space="PSUM"))
        psw = ctx.enter_context(tc.tile_pool(name="psw", bufs=1, space="PSUM"))

        identb = const.tile([P, P], bf16)
        make_identity(nc, identb)

        # PE clock-ramp warmup: dummy matmuls keep the PE busy from t~0 so
        # the ramp (3us of continuous execution in the HW/cost model) is
        # complete before the first real conv matmul. Results never read.
        warm = psw.tile([P, P], f32)
        for _ in range(NWARM):
            nc.tensor.matmul(warm, lhsT=identb, rhs=identb, start=True, stop=True)

        # sign weights, resident for the whole kernel; oc0 half first so the
        # first conv group can start as soon as possible
        wt = wpool.tile([P, 9, 2, C], f8, name="wt")
        nc.scalar.dma_start(out=wt[:, :, :, :P], in_=wq[:, :, :, :P])

        # row-chunk boundaries for the input loads (flat ranges, ends widened
        # to cover the guard elements)
        row_chunks = [(0, 15), (15, 30), (30, 44), (44, YP)]
        bounds = []
        for q, (r0, r1) in enumerate(row_chunks):
            f0 = 0 if q == 0 else _flat(r0, 0)
            f1 = FLAT if q == len(row_chunks) - 1 else _flat(r1, 0)
            bounds.append((f0, f1))

        def _one_image(img, first=False):
            # [cin_p, tag, cc, flat] fp8 padded channel-major image
            xp = xpool.tile([P, 2, 2, FLAT], f8, name="xp")
            for q, (f0, f1) in enumerate(bounds):
                for tag in range(2):
                    eng = nc.sync if tag == 0 else nc.scalar
                    eng.dma_start(
                        out=xp[:, tag, :, f0:f1],
                        in_=xq[img, tag, :, :, f0:f1].rearrange("c p f -> p c f"),
                    )
                if first and q == 0:
                    # oc1 weight half: after the first chunk so the first
                    # conv group isn't queued behind it on the DMA engines
                    nc.scalar.dma_start(
                        out=wt[:, :, :, P:], in_=wq[:, :, :, P:]
                    )

            for oc in range(2):
                ocmp = outp.tile([P, NPIX], bf16, name="ocmp")
                for t in range(NT):
                    ps = psc.tile([P, RB, XP], f32, name="ps")
                    psf = ps.rearrange("p r x -> p (r x)")
                    i = 0
                    for ky, kx in TAPS:
                        lhsT = wt[:, 3 * ky + kx, :, P * oc : P * (oc + 1)]
                        off = _flat(1 + RB * t + (ky - 1), kx - 1)
                        for tag in range(2):
                            nc.tensor.matmul(
                                psf,
                                lhsT=lhsT,
                                rhs=xp[:, tag, :, off : off + RB * XP],
                                start=(i == 0),
                                stop=(i == 17),
                                perf_mode=DR,
                            )
                            i += 1
                    nc.vector.tensor_copy(
                        out=ocmp[:, RB * W * t : RB * W * (t + 1)].rearrange(
                            "p (r w) -> p r w", w=W
                        ),
                        in_=ps[:, :, 1 : 1 + W],
                    )
                # split the store so the trailing transfer after the last
                # matmul is small; last chunk on HWDGE (sync) — lower latency
                # than the Pool SWDGE path
                nc.gpsimd.dma_start(
                    out=y[img, oc, :, : RB * W * (NT - 1)],
                    in_=ocmp[:, : RB * W * (NT - 1)],
                )
                nc.sync.dma_start(
                    out=y[img, oc, :, RB * W * (NT - 1) :],
                    in_=ocmp[:, RB * W * (NT - 1) :],
                )

        def _images():
            for img in range(ni):
                _one_image(img, first=(img == 0))

        if loops == 1:
            _images()
        else:
            with tc.For_i(0, loops, 1):
                _images()
    nc.compile()
    return nc


def get_bass(ni=NI, loops=1):
    key = (ni, loops)
    if key not in _cache:
        _cache[key] = _build_bass(ni, loops)
    return _cache[key]


def _prep_inputs(inputs, kernel):
    """Host-side shard prep: binarize weights, fp8 hi/lo split, channel-major
    padded layout."""
    import ml_dtypes

    E4 = ml_dtypes.float8_e4m3

    x = np.ascontiguousarray(inputs, dtype=np.float32)
    hi8 = x.astype(E4)
    lo8 = (x - hi8.astype(np.float32)).astype(E4)

    xq = np.zeros((NTOT, 2, 2, P, FLAT), dtype=E4)
    view = xq[..., 1 : 1 + YP * XP].reshape(NTOT, 2, 2, P, YP, XP)
    for tag, t8 in enumerate((hi8, lo8)):
        z = t8.reshape(NTOT, H, W, 2, P).transpose(0, 3, 4, 1, 2)
        view[:, tag, :, :, 1 : 1 + H, 1 : 1 + W] = z

    w = np.ascontiguousarray(kernel, dtype=np.float32)
    s = np.sign(np.clip(w, -1.0, 1.0))
    wqa = (
        s.reshape(3, 3, 2, P, C).transpose(3, 0, 1, 2, 4).reshape(P, 9, 2, C)
    ).astype(E4)
    return xq, wqa


def run(inputs, kernel, trace=False, **kw):
    from concourse.bass_utils import run_bass_kernel_spmd

    nc = get_bass()
    xq, wqa = _prep_inputs(inputs, kernel)
    in_maps = [
        {"xq": xq[i * NI : (i + 1) * NI], "wq": wqa} for i in range(NCORES)
    ]
    res = run_bass_kernel_spmd(nc, in_maps, core_ids=list(range(NCORES)),
                               trace=trace, **kw)
    yq = np.concatenate([r["y"] for r in res.results], axis=0)
    # [n, oc, p, pix] bf16 -> [n, pix, oc*128+p] fp32
    out = yq.transpose(0, 3, 1, 2).reshape(NTOT, NPIX, C).astype(np.float32)
    return out.reshape(NTOT, H, W, C), res


def kernel(**inputs):
    out, _ = run(inputs["inputs"], inputs["kernel"])
    return out


# revision 12
# speedup vs baseline: 1.0036x; 1.0036x over previous
"""BinaryConv2D forward on 8 Trainium2 NeuronCores.

out = conv2d_same(inputs, sign(clip(kernel)))   (NHWC, HWIO, 3x3, stride 1)

Sharding: data-parallel over batch (32 images -> 4 per core); the binarized
3x3x256x256 kernel is replicated (forward only, no gradient collective).

Strategy (v2 — fp8 DoubleRow):
  - Weights are +-1 after sign(), exactly representable in fp8e4. The input
    is split on the host into hi = fp8e4(x), lo = fp8e4(x - hi); two fp8
    passes reproduce ~7e-4 relative accuracy (gate is 2e-2) because the
    residual quantization error is (2^-4)^2 of |x|.
  - fp8e4 enables MatmulPerfMode.DoubleRow: the PE virtualizes to 128x256,
    contracting all 256 input channels in ONE matmul (lhsT [128,2,128],
    rhs [128,2,N]) at 2 MACs/cell/cycle — 4x fewer PE cycles than the bf16
    hi/lo baseline for the conv stream.
  - The host also pre-transposes each image to channel-major and embeds the
    SAME-padding in a flat padded layout: 58 rows x 58 cols (+1 guard elem at
    each end, 3366 per partition), zeros at the borders. Every conv tap
    (dy,dx) then reads one CONTIGUOUS shifted window of length 464 covering
    8 output rows — no on-device transposes, no per-tap row clipping, and
    the upload is 2 fp8 bytes/element instead of 4 (fp32).
  - Per (img, oc-half, 8-row block): one PSUM group of 18 accumulating
    DoubleRow matmuls (9 taps x {hi,lo}), out free = 464 fp32 (fits a 2KB
    bank). DVE evicts the 56 valid columns/row to bf16; the result is stored
    channel-major [oc, 128, 3136] and un-transposed to NHWC fp32 on the host.
  - ~36 warmup matmuls at t=0 cover the PE clock ramp while the weights and
    the first image chunks load.
"""

import numpy as np

P = 128
H = 56
W = 56
C = 256
XP = W + 1                   # padded row width (57): one zero col at x=-1.
                             # Reading one past the right edge lands on the
                             # NEXT row's pad col (also zero), so a single
                             # pad col covers both SAME-padding sides.
YP = H + 2                   # padded rows (58)
FLAT = YP * XP + 2           # flat padded image + 1 guard elem at each end
NCORES = 8
NTOT = 32
NI = NTOT // NCORES          # images per core
NPIX = H * W                 # 3136
RB = 8                       # output rows per psum block
NT = H // RB                 # 7 psum blocks
NWARM = 29                   # PE clock-ramp warmup matmuls

_cache = {}


def _flat(y, x):
    # flat index of padded coord (row y in 0..57, col x in 0..57)
    return 1 + y * XP + x


def _build_bass(ni=NI, loops=1):
    import concourse.bacc as bacc
    import concourse.mybir as mybir
    import concourse.tile as tile
    from concourse.masks import make_identity
    from contextlib import ExitStack

    f32 = mybir.dt.float32
    bf16 = mybir.dt.bfloat16
    f8 = mybir.dt.float8e4
    DR = mybir.MatmulPerfMode.DoubleRow

    nc = bacc.Bacc()
    # [img, tag(hi/lo), cc, cin_p, flat] fp8, host-padded (borders zero)
    xq = nc.dram_tensor("xq", [ni, 2, 2, P, FLAT], f8, kind="ExternalInput")
    # [cin_p, tap, cc, cout] fp8 sign weights, host-binarized
    wq = nc.dram_tensor("wq", [P, 9, 2, C], f8, kind="ExternalInput")
    # channel-major bf16 output; host un-transposes to NHWC fp32
    y = nc.dram_tensor("y", [ni, 2, P, NPIX], bf16, kind="ExternalOutput")

    TAPS = [(ky, kx) for ky in range(3) for kx in range(3)]

    with ExitStack() as ctx:
        tc = ctx.enter_context(tile.TileContext(nc))
        const = ctx.enter_context(tc.tile_pool(name="const", bufs=1))
        wpool = ctx.enter_context(tc.tile_pool(name="wpool", bufs=1))
        xpool = ctx.enter_context(tc.tile_pool(name="xpool", bufs=2))
        outp = ctx.enter_context(tc.tile_pool(name="outp", bufs=3))
        psc = ctx.enter_context(tc.tile_pool(name="psc", bufs=3, space="PSUM"))
        psw = ctx.enter_context(tc.tile_pool(name="psw", bufs=1, space="PSUM"))

        identb = const.tile([P, P], bf16)
        make_identity(nc, identb)

        # PE clock-ramp warmup: dummy matmuls keep the PE busy from t~0 so
        # the ramp (3us of continuous execution in the HW/cost model) is
        # complete before the first real conv matmul. Results never read.
        warm = psw.tile([P, P], f32)
        for _ in range(NWARM):
            nc.tensor.matmul(warm, lhsT=identb, rhs=identb, start=True, stop=True)

        # sign weights, resident for the whole kernel; oc0 half first so the
        # first conv group can start as soon as possible
        wt = wpool.tile([P, 9, 2, C], f8, name="wt")
        nc.scalar.dma_start(out=wt[:, :, :, :P], in_=wq[:, :, :, :P])

        # row-chunk boundaries for the input loads (flat ranges, ends widened
        # to cover the guard elements)
        row_chunks = [(0, 15), (15, 30), (30, 44), (44, YP)]
        bounds = []
        for q, (r0, r1) in enumerate(row_chunks):
            f0 = 0 if q == 0 else _flat(r0, 0)
            f1 = FLAT if q == len(row_chunks) - 1 else _flat(r1, 0)
            bounds.append((f0, f1))

        def _one_image(img, first=False):
            # [cin_p, tag, cc, flat] fp8 padded channel-major image
            xp = xpool.tile([P, 2, 2, FLAT], f8, name="xp")
            for q, (f0, f1) in enumerate(bounds):
                for tag in range(2):
                    eng = nc.sync if tag == 0 else nc.scalar
                    eng.dma_start(
                        out=xp[:, tag, :, f0:f1],
                        in_=xq[img, tag, :, :, f0:f1].rearrange("c p f -> p c f"),
                    )
                if first and q == 0:
                    # oc1 weight half: after the first chunk so the first
                    # conv group isn't queued behind it on the DMA engines
                    nc.scalar.dma_start(
                        out=wt[:, :, :, P:], in_=wq[:, :, :, P:]
                    )

            for oc in range(2):
                ocmp = outp.tile([P, NPIX], bf16, name="ocmp")
                for t in range(NT):
                    ps = psc.tile([P, RB, XP], f32, name="ps")
                    psf = ps.rearrange("p r x -> p (r x)")
                    i = 0
                    for ky, kx in TAPS:
                        lhsT = wt[:, 3 * ky + kx, :, P * oc : P * (oc + 1)]
                        off = _flat(1 + RB * t + (ky - 1), kx - 1)
                        for tag in range(2):
                            nc.tensor.matmul(
                                psf,
                                lhsT=lhsT,
                                rhs=xp[:, tag, :, off : off + RB * XP],
                                start=(i == 0),
                                stop=(i == 17),
                                perf_mode=DR,
                            )
                            i += 1
                    nc.vector.tensor_copy(
                        out=ocmp[:, RB * W * t : RB * W * (t + 1)].rearrange(
                            "p (r w) -> p r w", w=W
                        ),
                        in_=ps[:, :, 1 : 1 + W],
                    )
                # split the store so the trailing transfer after the last
                # matmul is small; last chunk on HWDGE (sync) — lower latency
                # than the Pool SWDGE path
                nc.gpsimd.dma_start(
                    out=y[img, oc, :, : RB * W * (NT - 1)],
                    in_=ocmp[:, : RB * W * (NT - 1)],
                )
                nc.sync.dma_start(
                    out=y[img, oc, :, RB * W * (NT - 1) :],
                    in_=ocmp[:, RB * W * (NT - 1) :],
                )

        def _images():
            for img in range(ni):
                _one_image(img, first=(img == 0))

        if loops == 1:
            _images()
        else:
            with tc.For_i(0, loops, 1):
                _images()
    nc.compile()
    return nc


def get_bass(ni=NI, loops=1):
    key = (ni, loops)
    if key not in _cache:
        _cache[key] = _build_bass(ni, loops)
    return _cache[key]


def _prep_inputs(inputs, kernel):
    """Host-side shard prep: binarize weights, fp8 hi/lo split, channel-major
    padded layout."""
    import ml_dtypes

    E4 = ml_dtypes.float8_e4m3

    x = np.ascontiguousarray(inputs, dtype=np.float32)
    hi8 = x.astype(E4)
    lo8 = (x - hi8.astype(np.float32)).astype(E4)

    xq = np.zeros((NTOT, 2, 2, P, FLAT), dtype=E4)
    view = xq[..., 1 : 1 + YP * XP].reshape(NTOT, 2, 2, P, YP, XP)
    for tag, t8 in enumerate((hi8, lo8)):
        z = t8.reshape(NTOT, H, W, 2, P).transpose(0, 3, 4, 1, 2)
        view[:, tag, :, :, 1 : 1 + H, 1 : 1 + W] = z

    w = np.ascontiguousarray(kernel, dtype=np.float32)
    s = np.sign(np.clip(w, -1.0, 1.0))
    wqa = (
        s.reshape(3, 3, 2, P, C).transpose(3, 0, 1, 2, 4).reshape(P, 9, 2, C)
    ).astype(E4)
    return xq, wqa


def run(inputs, kernel, trace=False, **kw):
    from concourse.bass_utils import run_bass_kernel_spmd

    nc = get_bass()
    xq, wqa = _prep_inputs(inputs, kernel)
    in_maps = [
        {"xq": xq[i * NI : (i + 1) * NI], "wq": wqa} for i in range(NCORES)
    ]
    res = run_bass_kernel_spmd(nc, in_maps, core_ids=list(range(NCORES)),
                               trace=trace, **kw)
    yq = np.concatenate([r["y"] for r in res.results], axis=0)
    # [n, oc, p, pix] bf16 -> [n, pix, oc*128+p] fp32
    out = yq.transpose(0, 3, 1, 2).reshape(NTOT, NPIX, C).astype(np.float32)
    return out.reshape(NTOT, H, W, C), res


def kernel(**inputs):
    out, _ = run(inputs["inputs"], inputs["kernel"])
    return out
